# revision 27
# baseline (speedup 1.0000x reference)
"""Trainium2 Bass kernel for a 3-layer GCN (nn_BaselineGCN).

Strategy (8 NeuronCores, node partitioning by dst):
  - Host: compute deg/dis, partition edges by owner of dst (6250 nodes/core,
    padded to 6272), sort by (dst-window, src-half), build int16 gather
    indices (full node table split into two 25088-row halves so indices fit
    int16) plus per-edge local-dst values for one-hot construction.
  - Device, per layer (bf16 tables, fp32 accumulation):
      * data-parallel matmul  Zs_own = dis ⊙ (H_own @ W)        (TensorE)
      * AllGather Zs chunks -> full 50176-row bf16 table in HBM (collective)
      * per 128-dst window: dma_gather source rows (memory-bound part),
        one-hot(dst_local) built on DVE via is_equal vs iota, segment-sum
        via PE matmul accumulation in PSUM with one extra I @ Zs_own_w
        matmul for the (A+I) self term, epilogue relu(dis ⊙ acc + b),
        TensorE transpose -> next H^T kept resident in SBUF.
  - Layer 3 (64 outputs) runs on a 128-wide bf16 table (upper 64 cols
    garbage, excluded by slicing) so the whole edge path is uniform bf16.
"""
import sys
import os

sys.path.insert(0, "/opt/trn_rl_repo")

import numpy as np

NC_CORES = 8
GMAX = 8  # max groups (=1024 indices) per dma_gather call
GATH_BUFS = 8  # gather-tile pool depth (first GATH_BUFS windows are memset)


def _cdiv(a, b):
    return (a + b - 1) // b


# ---------------------------------------------------------------------------
# Host-side preprocessing
# ---------------------------------------------------------------------------
def preprocess(edge_index, N):
    src = np.asarray(edge_index[0], dtype=np.int64)
    dst = np.asarray(edge_index[1], dtype=np.int64)
    deg = np.bincount(dst, minlength=N).astype(np.float32) + np.float32(1.0)
    dis = (np.float32(1.0) / np.sqrt(deg)).astype(np.float32)

    CH = N // NC_CORES
    NWIN = _cdiv(CH, 128)
    CHP = NWIN * 128
    # split each core's chunk into A (windows 0..NWA-1) and B (the rest) so
    # the AllGather of A can fire mid-phase; int16 gather indices address
    # each half-table separately.
    # A as large as int16 gather indices allow (NC*HA <= 32768) so the
    # exposed tail AllGather over B is as small as possible
    NWA = min(NWIN - 1, 32768 // (NC_CORES * 128)) if NWIN > 1 else NWIN
    HA = NWA * 128            # rows per core in table A
    HB = CHP - HA             # rows per core in table B (may be 0 if NWIN==1)
    src_c = src // CH         # owning core of each src node
    src_o = src % CH          # offset within core

    counts = np.zeros((NC_CORES, NWIN, 2), dtype=np.int64)
    percore = []
    for c in range(NC_CORES):
        sel = (dst >= c * CH) & (dst < (c + 1) * CH)
        sc, so = src_c[sel], src_o[sel]
        ed = dst[sel] - c * CH
        w = ed >> 7
        h = (so >= HA).astype(np.int64)
        eidx = np.where(h == 0, sc * HA + so, sc * HB + (so - HA))
        order = np.lexsort((ed, h, w))
        eidx, ed, w, h = eidx[order], ed[order], w[order], h[order]
        np.add.at(counts[c], (w, h), 1)
        percore.append((eidx, ed, w, h))

    G = _cdiv(counts, 128).max(axis=0)  # [NWIN, 2]

    import ml_dtypes

    # shared call schedule: per (window, half), gather calls of <=GMAX groups
    calls = []  # (wi, hi, g0, gc)
    for wi in range(NWIN):
        for hi in range(2):
            g0 = 0
            while g0 < G[wi, hi]:
                gc = min(GMAX, G[wi, hi] - g0)
                calls.append((wi, hi, g0, gc))
                g0 += gc

    cores = []
    for c in range(NC_CORES):
        eidx, ed, w, h = percore[c]
        idx_parts, dstl_parts = [], []
        pos = 0
        for wi in range(NWIN):
            for hi in range(2):
                n = counts[c, wi, hi]
                g = G[wi, hi]
                seg_idx = np.full(g * 128, -1, dtype=np.int16)
                seg_dstl = np.full(g * 128, 255.0, dtype=np.float32)
                if n:
                    seg_idx[:n] = eidx[pos:pos + n].astype(np.int16)
                    seg_dstl[:n] = (ed[pos:pos + n] - wi * 128).astype(np.float32)
                    pos += n
                idx_parts.append(seg_idx)
                dstl_parts.append(seg_dstl)
        idx_all = np.concatenate(idx_parts)
        dstl_all = np.concatenate(dstl_parts)
        TOT_G = len(idx_all) // 128

        # per-call valid counts; empty calls keep one dummy idx (0) because a
        # zero-valid gather is undefined
        seg_base = {}
        pos2 = 0
        for wi in range(NWIN):
            for hi in range(2):
                seg_base[(wi, hi)] = pos2
                pos2 += G[wi, hi] * 128
        ncounts = np.zeros(len(calls), dtype=np.int32)
        for k, (wi, hi, g0, gc) in enumerate(calls):
            n = int(counts[c, wi, hi])
            v = min(max(n - g0 * 128, 0), gc * 128)
            if v == 0:
                idx_all[seg_base[(wi, hi)] + g0 * 128] = 0
                v = 1
            ncounts[k] = v

        # device layouts
        idx_tiled = np.tile(idx_all.reshape(-1, 16).T, (8, 1)).copy()
        # dstl: [128 edge-slot, TOT_G] bf16
        dstl_tiled = np.ascontiguousarray(
            dstl_all.reshape(TOT_G, 128).T).astype(ml_dtypes.bfloat16)
        d = np.ones(CHP, np.float32)
        d[:CH] = dis[c * CH:(c + 1) * CH]
        dis_win = np.ascontiguousarray(d.reshape(NWIN, 128).T)
        cores.append(dict(idx=idx_tiled, dstl=dstl_tiled, dis_win=dis_win,
                          ncounts=ncounts))
    return dis, G, cores, CH, NWIN, CHP, NWA, len(calls)


# ---------------------------------------------------------------------------
# Bass program
# ---------------------------------------------------------------------------
def build_program(DIN, DRS, DTS, G, NWIN, CHP, NWA, TOT_IDX, TOT_G,
                  G_CAP, NCALLS, biases_nonzero):
    """DRS: real per-layer output dims [256,256,64];
    DTS: padded table dims [256,256,128]."""
    from concourse import bacc, bass, tile, mybir

    f32 = mybir.dt.float32
    bf16 = mybir.dt.bfloat16
    i16 = mybir.dt.int16
    ADD = mybir.AluOpType.add
    EQ = mybir.AluOpType.is_equal
    CPY = mybir.ActivationFunctionType.Copy
    NL = len(DRS)

    nc = bacc.Bacc("TRN2", target_bir_lowering=False, debug=False,
                   enable_asserts=False, num_devices=NC_CORES,
                   num_swdge_queues=4, dynamic_dma_scratch_size=32768)

    # --- I/O tensors ---
    xT_d = nc.dram_tensor("xT", [DIN, CHP], bf16, kind="ExternalInput")
    W_d = [nc.dram_tensor(f"W{i}", [DRS[i - 1] if i else DIN, DRS[i]], bf16,
                          kind="ExternalInput") for i in range(NL)]
    bias_d = [nc.dram_tensor(f"bias{i}", [128, DRS[i]], f32,
                             kind="ExternalInput") for i in range(NL)]
    idx_d = nc.dram_tensor("idx", [128, TOT_IDX // 16], i16, kind="ExternalInput")
    dstl_d = nc.dram_tensor("dstl", [128, TOT_G], bf16, kind="ExternalInput")
    iotag_d = nc.dram_tensor("iotag", [128, 128 * G_CAP], bf16,
                             kind="ExternalInput")
    dis_d = nc.dram_tensor("dis_win", [128, NWIN], f32, kind="ExternalInput")
    ident_d = nc.dram_tensor("ident", [128, 128], bf16, kind="ExternalInput")
    cnt_d = nc.dram_tensor("ncounts", [1, NCALLS], mybir.dt.int32,
                           kind="ExternalInput")
    out_d = nc.dram_tensor("out", [CHP, DRS[-1]], f32, kind="ExternalOutput")

    with tile.TileContext(nc) as tc:
        with (
            tc.tile_pool(name="const", bufs=1) as constp,
            tc.tile_pool(name="ht", bufs=1) as htp,
            tc.tile_pool(name="wts", bufs=2) as wtsp,
            tc.tile_pool(name="zs", bufs=3) as zsp,
            tc.tile_pool(name="gath", bufs=GATH_BUFS) as gathp,
            tc.tile_pool(name="oh", bufs=4) as ohp,
            tc.tile_pool(name="epi", bufs=3) as epip,
            tc.tile_pool(name="psz", bufs=2, space="PSUM") as pszp,
            tc.tile_pool(name="psw", bufs=3, space="PSUM") as pswp,
            tc.tile_pool(name="pst", bufs=2, space="PSUM") as pstp,
            tc.tile_pool(name="dram", bufs=1, space="DRAM") as dramp,
        ):
            # warm up the one-time collectives barrier immediately so it
            # overlaps the z-phase instead of delaying the first AllGather
            warm_in = dramp.tile([1, 128], bf16, tag="warm_in",
                                 name="warm_in")
            warm_out = dramp.tile([NC_CORES, 128], bf16, tag="warm_out",
                                  addr_space="Shared", name="warm_out")
            nc.gpsimd.collective_compute(
                "AllGather", bass.mybir.AluOpType.bypass,
                replica_groups=[list(range(NC_CORES))],
                ins=[warm_in[:]], outs=[warm_out.opt()])

            # --- persistent SBUF constants ---
            idx_t = constp.tile([128, TOT_IDX // 16], i16, tag="idx")
            nc.sync.dma_start(idx_t[:], idx_d[:])
            dstl_t = constp.tile([128, TOT_G], bf16, tag="dstl")
            nc.sync.dma_start(dstl_t[:], dstl_d[:])
            iotag_t = constp.tile([128, 128 * G_CAP], bf16, tag="iotag")
            nc.sync.dma_start(iotag_t[:], iotag_d[:])
            dis_t = constp.tile([128, NWIN], f32, tag="dis")
            nc.sync.dma_start(dis_t[:], dis_d[:])
            ident_t = constp.tile([128, 128], bf16, tag="ident")
            nc.sync.dma_start(ident_t[:], ident_d[:])
            cnt_t = constp.tile([1, NCALLS], mybir.dt.int32, tag="cnt")
            nc.sync.dma_start(cnt_t[:], cnt_d[:])
            cnt_regs = [nc.gpsimd.alloc_register(f"gcnt{i}") for i in range(4)]
            bias_t = []
            for i in range(NL):
                if biases_nonzero[i]:
                    bt = constp.tile([128, DRS[i]], f32, tag=f"bias{i}")
                    nc.sync.dma_start(bt[:], bias_d[i][:])
                    bias_t.append(bt)
                else:
                    bias_t.append(None)

            # --- H^T SBUF-resident double buffer: [k][128, CHP] bf16 ---
            KT0 = DIN // 128
            ht_cur = [htp.tile([128, CHP], bf16, tag=f"htA{k}",
                               name=f"htA{k}") for k in range(KT0)]
            XCH = _cdiv(CHP, 4 * 128) * 128
            for k in range(KT0):
                for x0 in range(0, CHP, XCH):
                    x1 = min(x0 + XCH, CHP)
                    nc.sync.dma_start(
                        ht_cur[k][:, x0:x1],
                        xT_d[k * 128:(k + 1) * 128, x0:x1])
            ht_nxt = [htp.tile([128, CHP], bf16, tag=f"htB{k}",
                               name=f"htB{k}") for k in range(KT0)]

            zs_own = [dramp.tile([CHP, DTS[i]], bf16, tag=f"zso{i}",
                                 name=f"zs_own{i}") for i in range(NL)]
            HA = NWA * 128
            HB = CHP - HA
            zs_fullA = [dramp.tile([NC_CORES * HA, DTS[i]], bf16,
                                   tag=f"zsfA{i}", addr_space="Shared",
                                   name=f"zs_fullA{i}") for i in range(NL)]
            zs_fullB = [dramp.tile([NC_CORES * HB, DTS[i]], bf16,
                                   tag=f"zsfB{i}", addr_space="Shared",
                                   name=f"zs_fullB{i}") for i in range(NL)]

            RG = [list(range(NC_CORES))]

            def emit_z_tile(li, t, lhs_tiles):
                """Z-matmul + scale for node-tile t of layer li."""
                Dr = DRS[li]
                Dt = DTS[li]
                KT = DIN // 128 if li == 0 else DRS[li - 1] // 128
                psz = pszp.tile([128, Dr], f32, tag="psz", name="psz")
                for k in range(KT):
                    nc.tensor.matmul(psz[:],
                                     lhs_tiles[k][:, t * 128:(t + 1) * 128],
                                     wk[li][k][:],
                                     start=(k == 0), stop=(k == KT - 1))
                zst = zsp.tile([128, Dt], bf16, tag="zst", name="zst")
                nc.scalar.activation(zst[:, :Dr], psz[:], CPY,
                                     scale=dis_t[:, t:t + 1])
                nc.sync.dma_start(zs_own[li][t * 128:(t + 1) * 128, :Dr],
                                  zst[:, :Dr])

            def emit_ag(li, half):
                if half == 0:
                    nc.gpsimd.collective_compute(
                        "AllGather", bass.mybir.AluOpType.bypass,
                        replica_groups=RG,
                        ins=[zs_own[li][:HA, :]],
                        outs=[zs_fullA[li].opt()])
                else:
                    nc.gpsimd.collective_compute(
                        "AllGather", bass.mybir.AluOpType.bypass,
                        replica_groups=RG,
                        ins=[zs_own[li][HA:, :]],
                        outs=[zs_fullB[li].opt()])

            # weight tiles for every layer, loaded up front (small)
            wk = []
            for li in range(NL):
                KT = DIN // 128 if li == 0 else DRS[li - 1] // 128
                wkl = []
                for k in range(KT):
                    wt_ = wtsp.tile([128, DRS[li]], bf16, tag=f"wk{li}_{k}",
                                    name=f"wk{li}_{k}")
                    nc.sync.dma_start(wt_[:], W_d[li][k * 128:(k + 1) * 128, :])
                    wkl.append(wt_)
                wk.append(wkl)

            # ---- layer-0 z-phase + split AllGather ----
            for t in range(NWIN):
                emit_z_tile(0, t, ht_cur)
                if t == NWA - 1:
                    emit_ag(0, 0)
            emit_ag(0, 1)

            # per-(window,half) idx/call offsets (stream: w-major, A then B)
            goffs = {}
            coffs = {}
            go = 0
            ci = 0
            for w in range(NWIN):
                for hh in range(2):
                    goffs[(w, hh)] = go
                    coffs[(w, hh)] = ci
                    go += int(G[w, hh])
                    ci += _cdiv(int(G[w, hh]), GMAX)

            # ---- main loop: gather/aggregate layer li; z + AG of li+1
            #      interleaved so the collectives hide under the gathers.
            #      The first GATH_BUFS windows' A-half calls are emitted
            #      before any B-half call: the in-order gpsimd queue then
            #      generates A descriptors while the B-table AllGather is
            #      still in flight instead of stalling behind it. ----
            for li in range(NL):
                Dr = DRS[li]
                Dt = DTS[li]
                qrr = li * 3  # decorrelate queue phase across layers

                def emit_calls(w, half, wt, Gl):
                    nonlocal qrr
                    tbl = zs_fullA[li] if half == 0 else zs_fullB[li]
                    gcnt = int(G[w, half])
                    gbase = 0 if half == 0 else Gl
                    g0 = 0
                    ci2 = coffs[(w, half)]
                    while g0 < gcnt:
                        gc = min(GMAX, gcnt - g0)
                        reg = cnt_regs[qrr % 4]
                        nc.gpsimd.reg_load(reg, cnt_t[0:1, ci2:ci2 + 1])
                        nc.gpsimd.dma_gather(
                            wt[:, gbase + g0:gbase + g0 + gc, :],
                            tbl[:],
                            idx_t[:, (goffs[(w, half)] + g0) * 8:
                                  (goffs[(w, half)] + g0 + gc) * 8],
                            num_idxs=gc * 128,
                            num_idxs_reg=reg,
                            elem_size=Dt,
                            queue_num=qrr % 4,
                        )
                        qrr += 1
                        ci2 += 1
                        g0 += gc

                PRE = min(GATH_BUFS, NWIN)
                tiles = {}
                for w in range(PRE):
                    wt = gathp.tile([128, G_CAP, Dt], bf16, tag="gather",
                                    name="wt")
                    if li == 0:
                        # first pool rotation: clear so slots skipped by
                        # short gathers never hold NaN bit patterns
                        nc.vector.memset(wt[:], 0.0)
                    tiles[w] = wt
                    emit_calls(w, 0, wt, int(G[w, 0]))
                for w in range(NWIN):
                    Gl, Gh = int(G[w, 0]), int(G[w, 1])
                    Gt = Gl + Gh
                    assert Gt > 0, "empty window unsupported"
                    if w < PRE:
                        wt = tiles.pop(w)
                    else:
                        wt = gathp.tile([128, G_CAP, Dt], bf16, tag="gather",
                                        name="wt")
                        emit_calls(w, 0, wt, Gl)
                    emit_calls(w, 1, wt, Gl)
                    # one-hot: oh[p, j, g] = (dstl[p, g] == j)
                    g_off = goffs[(w, 0)]
                    oh = ohp.tile([128, 128, Gt], bf16, tag="oh", name="oh")
                    nc.vector.tensor_tensor(
                        oh[:],
                        dstl_t[:, g_off:g_off + Gt].unsqueeze(1)
                            .broadcast_to((128, 128, Gt)),
                        iotag_t[:].rearrange("p (j g) -> p j g", g=G_CAP)
                            [:, :, :Gt],
                        op=EQ,
                    )
                    # z rows for the self term (bf16, local DRAM)
                    zw = zsp.tile([128, Dr], bf16, tag="zw", name="zw")
                    nc.scalar.dma_start(
                        zw[:], zs_own[li][w * 128:(w + 1) * 128, :Dr])
                    psw = pswp.tile([128, Dt], f32, tag="psw", name="psw")
                    for g in range(Gt):
                        nc.tensor.matmul(psw[:], oh[:, :, g], wt[:, g, :],
                                         start=(g == 0), stop=False)
                    # self term: psw[j] += zs_own[w*128+j] (epilogue's dis
                    # scale turns this into the dis^2 z contribution)
                    nc.tensor.matmul(psw[:, :Dr], ident_t[:], zw[:],
                                     start=False, stop=True)
                    if bias_t[li] is not None:
                        t1 = epip.tile([128, Dr], f32, tag="t1", name="t1")
                        nc.vector.tensor_tensor(t1[:], psw[:, :Dr],
                                                bias_t[li][:], op=ADD)
                        src_ap = t1[:]
                    else:
                        src_ap = psw[:, :Dr]
                    if li < NL - 1:
                        h2 = epip.tile([128, Dr], bf16, tag="h2", name="h2")
                        nc.scalar.activation(
                            h2[:], src_ap,
                            bass.mybir.ActivationFunctionType.Relu,
                            scale=dis_t[:, w:w + 1])
                        for k in range(Dr // 128):
                            pst = pstp.tile([128, 128], bf16, tag="pst",
                                            name="pst")
                            nc.tensor.transpose(
                                pst[:], h2[:, k * 128:(k + 1) * 128],
                                ident_t[:])
                            nc.vector.tensor_copy(
                                ht_nxt[k][:, w * 128:(w + 1) * 128], pst[:])
                        # interleaved z for layer li+1 (its H^T tile-w is
                        # ready now); AG halves fire mid-phase
                        emit_z_tile(li + 1, w, ht_nxt)
                        if w == NWA - 1:
                            emit_ag(li + 1, 0)
                    else:
                        h2o = epip.tile([128, Dr], f32, tag="h2o", name="h2o")
                        nc.scalar.activation(h2o[:], src_ap, CPY,
                                             scale=dis_t[:, w:w + 1])
                        nc.sync.dma_start(out_d[w * 128:(w + 1) * 128, :],
                                          h2o[:])
                if li < NL - 1:
                    emit_ag(li + 1, 1)
                    ht_cur, ht_nxt = ht_nxt, ht_cur
    nc.compile()
    return nc


# ---------------------------------------------------------------------------
# Entry point
# ---------------------------------------------------------------------------
def kernel(x, edge_index, W1, b1, W2, b2, W3, b3):
    from concourse.bass_utils import run_bass_kernel_spmd
    import ml_dtypes

    bfnp = ml_dtypes.bfloat16
    x = np.asarray(x, dtype=np.float32)
    Ws = [np.asarray(w, dtype=np.float32) for w in (W1, W2, W3)]
    bs = [np.asarray(b, dtype=np.float32) for b in (b1, b2, b3)]

    N, DIN = x.shape
    DRS = [w.shape[1] for w in Ws]
    DTS = [max(d, 128) for d in DRS]
    NL = 3

    dis, G, cores, CH, NWIN, CHP, NWA, NCALLS = preprocess(edge_index, N)
    TOT_IDX = cores[0]["idx"].shape[1] * 16
    TOT_G = cores[0]["dstl"].shape[1]
    G_CAP = int((G[:, 0] + G[:, 1]).max())
    biases_nonzero = [bool(np.any(b != 0)) for b in bs]

    nc = build_program(DIN, DRS, DTS, G, NWIN, CHP, NWA, TOT_IDX, TOT_G,
                       G_CAP, NCALLS, biases_nonzero)

    ident = np.eye(128, dtype=bfnp)
    # iotag[p, j*G_CAP + g] = j
    iotag = np.tile(np.repeat(np.arange(128), G_CAP).astype(bfnp), (128, 1))
    in_maps = []
    for c in range(NC_CORES):
        xT = np.zeros((DIN, CHP), bfnp)
        xT[:, :CH] = x[c * CH:(c + 1) * CH].T.astype(bfnp)
        m = {
            "xT": xT,
            "idx": cores[c]["idx"],
            "dstl": cores[c]["dstl"],
            "iotag": iotag,
            "dis_win": cores[c]["dis_win"],
            "ident": ident,
            "ncounts": cores[c]["ncounts"][None, :],
        }
        for i in range(NL):
            m[f"W{i}"] = Ws[i].astype(bfnp)
            m[f"bias{i}"] = np.tile(bs[i][None, :], (128, 1))
        in_maps.append(m)

    trace = bool(int(os.environ.get("GCN_TRACE", "0")))
    res = run_bass_kernel_spmd(nc, in_maps, core_ids=list(range(NC_CORES)),
                               trace=trace)
    kernel.last_results = res
    out = np.concatenate([res.results[c]["out"][:CH] for c in range(NC_CORES)],
                         axis=0)
    return out.astype(np.float32)



# revision 28
# speedup vs baseline: 1.0056x; 1.0056x over previous
"""Trainium2 Bass kernel for a 3-layer GCN (nn_BaselineGCN).

Strategy (8 NeuronCores, node partitioning by dst):
  - Host: compute deg/dis, partition edges by owner of dst (6250 nodes/core,
    padded to 6272), sort by (dst-window, src-half), build int16 gather
    indices (full node table split into two 25088-row halves so indices fit
    int16) plus per-edge local-dst values for one-hot construction.
  - Device, per layer (bf16 tables, fp32 accumulation):
      * data-parallel matmul  Zs_own = dis ⊙ (H_own @ W)        (TensorE)
      * AllGather Zs chunks -> full 50176-row bf16 table in HBM (collective)
      * per 128-dst window: dma_gather source rows (memory-bound part),
        one-hot(dst_local) built on DVE via is_equal vs iota, segment-sum
        via PE matmul accumulation in PSUM with one extra I @ Zs_own_w
        matmul for the (A+I) self term, epilogue relu(dis ⊙ acc + b),
        TensorE transpose -> next H^T kept resident in SBUF.
  - Layer 3 (64 outputs) runs on a 128-wide bf16 table (upper 64 cols
    garbage, excluded by slicing) so the whole edge path is uniform bf16.
"""
import sys
import os

sys.path.insert(0, "/opt/trn_rl_repo")

import numpy as np

NC_CORES = 8
GMAX = 8  # max groups (=1024 indices) per dma_gather call
GATH_BUFS = 8  # gather-tile pool depth (first GATH_BUFS windows are memset)


def _cdiv(a, b):
    return (a + b - 1) // b


# ---------------------------------------------------------------------------
# Host-side preprocessing
# ---------------------------------------------------------------------------
def preprocess(edge_index, N):
    src = np.asarray(edge_index[0], dtype=np.int64)
    dst = np.asarray(edge_index[1], dtype=np.int64)
    deg = np.bincount(dst, minlength=N).astype(np.float32) + np.float32(1.0)
    dis = (np.float32(1.0) / np.sqrt(deg)).astype(np.float32)

    CH = N // NC_CORES
    NWIN = _cdiv(CH, 128)
    CHP = NWIN * 128
    # split each core's chunk into A (windows 0..NWA-1) and B (the rest) so
    # the AllGather of A can fire mid-phase; int16 gather indices address
    # each half-table separately.
    # A as large as int16 gather indices allow (NC*HA <= 32768) so the
    # exposed tail AllGather over B is as small as possible
    NWA = min(NWIN - 1, 32768 // (NC_CORES * 128)) if NWIN > 1 else NWIN
    HA = NWA * 128            # rows per core in table A
    HB = CHP - HA             # rows per core in table B (may be 0 if NWIN==1)
    src_c = src // CH         # owning core of each src node
    src_o = src % CH          # offset within core

    counts = np.zeros((NC_CORES, NWIN, 2), dtype=np.int64)
    percore = []
    for c in range(NC_CORES):
        sel = (dst >= c * CH) & (dst < (c + 1) * CH)
        sc, so = src_c[sel], src_o[sel]
        ed = dst[sel] - c * CH
        w = ed >> 7
        h = (so >= HA).astype(np.int64)
        eidx = np.where(h == 0, sc * HA + so, sc * HB + (so - HA))
        order = np.lexsort((ed, h, w))
        eidx, ed, w, h = eidx[order], ed[order], w[order], h[order]
        np.add.at(counts[c], (w, h), 1)
        percore.append((eidx, ed, w, h))

    G = _cdiv(counts, 128).max(axis=0)  # [NWIN, 2]

    import ml_dtypes

    # shared call schedule: per (window, half), gather calls of <=GMAX groups
    calls = []  # (wi, hi, g0, gc)
    for wi in range(NWIN):
        for hi in range(2):
            g0 = 0
            while g0 < G[wi, hi]:
                gc = min(GMAX, G[wi, hi] - g0)
                calls.append((wi, hi, g0, gc))
                g0 += gc

    cores = []
    for c in range(NC_CORES):
        eidx, ed, w, h = percore[c]
        idx_parts, dstl_parts = [], []
        pos = 0
        for wi in range(NWIN):
            for hi in range(2):
                n = counts[c, wi, hi]
                g = G[wi, hi]
                seg_idx = np.full(g * 128, -1, dtype=np.int16)
                seg_dstl = np.full(g * 128, 255.0, dtype=np.float32)
                if n:
                    seg_idx[:n] = eidx[pos:pos + n].astype(np.int16)
                    seg_dstl[:n] = (ed[pos:pos + n] - wi * 128).astype(np.float32)
                    pos += n
                idx_parts.append(seg_idx)
                dstl_parts.append(seg_dstl)
        idx_all = np.concatenate(idx_parts)
        dstl_all = np.concatenate(dstl_parts)
        TOT_G = len(idx_all) // 128

        # per-call valid counts; empty calls keep one dummy idx (0) because a
        # zero-valid gather is undefined
        seg_base = {}
        pos2 = 0
        for wi in range(NWIN):
            for hi in range(2):
                seg_base[(wi, hi)] = pos2
                pos2 += G[wi, hi] * 128
        ncounts = np.zeros(len(calls), dtype=np.int32)
        for k, (wi, hi, g0, gc) in enumerate(calls):
            n = int(counts[c, wi, hi])
            v = min(max(n - g0 * 128, 0), gc * 128)
            if v == 0:
                idx_all[seg_base[(wi, hi)] + g0 * 128] = 0
                v = 1
            ncounts[k] = v

        # device layouts
        idx_tiled = np.tile(idx_all.reshape(-1, 16).T, (8, 1)).copy()
        # dstl: [128 edge-slot, TOT_G] bf16
        dstl_tiled = np.ascontiguousarray(
            dstl_all.reshape(TOT_G, 128).T).astype(ml_dtypes.bfloat16)
        d = np.ones(CHP, np.float32)
        d[:CH] = dis[c * CH:(c + 1) * CH]
        dis_win = np.ascontiguousarray(d.reshape(NWIN, 128).T)
        cores.append(dict(idx=idx_tiled, dstl=dstl_tiled, dis_win=dis_win,
                          ncounts=ncounts))
    return dis, G, cores, CH, NWIN, CHP, NWA, len(calls)


# ---------------------------------------------------------------------------
# Bass program
# ---------------------------------------------------------------------------
def build_program(DIN, DRS, DTS, G, NWIN, CHP, NWA, TOT_IDX, TOT_G,
                  G_CAP, NCALLS, biases_nonzero):
    """DRS: real per-layer output dims [256,256,64];
    DTS: padded table dims [256,256,128]."""
    from concourse import bacc, bass, tile, mybir

    f32 = mybir.dt.float32
    bf16 = mybir.dt.bfloat16
    i16 = mybir.dt.int16
    ADD = mybir.AluOpType.add
    EQ = mybir.AluOpType.is_equal
    CPY = mybir.ActivationFunctionType.Copy
    NL = len(DRS)

    nc = bacc.Bacc("TRN2", target_bir_lowering=False, debug=False,
                   enable_asserts=False, num_devices=NC_CORES,
                   num_swdge_queues=4, dynamic_dma_scratch_size=32768)

    # --- I/O tensors ---
    xT_d = nc.dram_tensor("xT", [DIN, CHP], bf16, kind="ExternalInput")
    W_d = [nc.dram_tensor(f"W{i}", [DRS[i - 1] if i else DIN, DRS[i]], bf16,
                          kind="ExternalInput") for i in range(NL)]
    bias_d = [nc.dram_tensor(f"bias{i}", [128, DRS[i]], f32,
                             kind="ExternalInput") for i in range(NL)]
    idx_d = nc.dram_tensor("idx", [128, TOT_IDX // 16], i16, kind="ExternalInput")
    dstl_d = nc.dram_tensor("dstl", [128, TOT_G], bf16, kind="ExternalInput")
    iotag_d = nc.dram_tensor("iotag", [128, 128 * G_CAP], bf16,
                             kind="ExternalInput")
    dis_d = nc.dram_tensor("dis_win", [128, NWIN], f32, kind="ExternalInput")
    ident_d = nc.dram_tensor("ident", [128, 128], bf16, kind="ExternalInput")
    cnt_d = nc.dram_tensor("ncounts", [1, NCALLS], mybir.dt.int32,
                           kind="ExternalInput")
    out_d = nc.dram_tensor("out", [CHP, DRS[-1]], f32, kind="ExternalOutput")

    with tile.TileContext(nc) as tc:
        with (
            tc.tile_pool(name="const", bufs=1) as constp,
            tc.tile_pool(name="ht", bufs=1) as htp,
            tc.tile_pool(name="wts", bufs=2) as wtsp,
            tc.tile_pool(name="zs", bufs=3) as zsp,
            tc.tile_pool(name="gath", bufs=GATH_BUFS) as gathp,
            tc.tile_pool(name="oh", bufs=4) as ohp,
            tc.tile_pool(name="epi", bufs=3) as epip,
            tc.tile_pool(name="psz", bufs=2, space="PSUM") as pszp,
            tc.tile_pool(name="psw", bufs=3, space="PSUM") as pswp,
            tc.tile_pool(name="pst", bufs=2, space="PSUM") as pstp,
            tc.tile_pool(name="dram", bufs=1, space="DRAM") as dramp,
        ):
            # --- persistent SBUF constants ---
            idx_t = constp.tile([128, TOT_IDX // 16], i16, tag="idx")
            nc.sync.dma_start(idx_t[:], idx_d[:])
            dstl_t = constp.tile([128, TOT_G], bf16, tag="dstl")
            nc.sync.dma_start(dstl_t[:], dstl_d[:])
            iotag_t = constp.tile([128, 128 * G_CAP], bf16, tag="iotag")
            nc.sync.dma_start(iotag_t[:], iotag_d[:])
            dis_t = constp.tile([128, NWIN], f32, tag="dis")
            nc.sync.dma_start(dis_t[:], dis_d[:])
            ident_t = constp.tile([128, 128], bf16, tag="ident")
            nc.sync.dma_start(ident_t[:], ident_d[:])
            cnt_t = constp.tile([1, NCALLS], mybir.dt.int32, tag="cnt")
            nc.sync.dma_start(cnt_t[:], cnt_d[:])
            cnt_regs = [nc.gpsimd.alloc_register(f"gcnt{i}") for i in range(4)]
            bias_t = []
            for i in range(NL):
                if biases_nonzero[i]:
                    bt = constp.tile([128, DRS[i]], f32, tag=f"bias{i}")
                    nc.sync.dma_start(bt[:], bias_d[i][:])
                    bias_t.append(bt)
                else:
                    bias_t.append(None)

            # --- H^T SBUF-resident double buffer: [k][128, CHP] bf16 ---
            KT0 = DIN // 128
            ht_cur = [htp.tile([128, CHP], bf16, tag=f"htA{k}",
                               name=f"htA{k}") for k in range(KT0)]
            XCH = _cdiv(CHP, 4 * 128) * 128
            for k in range(KT0):
                for x0 in range(0, CHP, XCH):
                    x1 = min(x0 + XCH, CHP)
                    nc.sync.dma_start(
                        ht_cur[k][:, x0:x1],
                        xT_d[k * 128:(k + 1) * 128, x0:x1])
            ht_nxt = [htp.tile([128, CHP], bf16, tag=f"htB{k}",
                               name=f"htB{k}") for k in range(KT0)]

            zs_own = [dramp.tile([CHP, DTS[i]], bf16, tag=f"zso{i}",
                                 name=f"zs_own{i}") for i in range(NL)]
            HA = NWA * 128
            HB = CHP - HA
            zs_fullA = [dramp.tile([NC_CORES * HA, DTS[i]], bf16,
                                   tag=f"zsfA{i}", addr_space="Shared",
                                   name=f"zs_fullA{i}") for i in range(NL)]
            zs_fullB = [dramp.tile([NC_CORES * HB, DTS[i]], bf16,
                                   tag=f"zsfB{i}", addr_space="Shared",
                                   name=f"zs_fullB{i}") for i in range(NL)]

            RG = [list(range(NC_CORES))]

            def emit_z_tile(li, t, lhs_tiles):
                """Z-matmul + scale for node-tile t of layer li."""
                Dr = DRS[li]
                Dt = DTS[li]
                KT = DIN // 128 if li == 0 else DRS[li - 1] // 128
                psz = pszp.tile([128, Dr], f32, tag="psz", name="psz")
                for k in range(KT):
                    nc.tensor.matmul(psz[:],
                                     lhs_tiles[k][:, t * 128:(t + 1) * 128],
                                     wk[li][k][:],
                                     start=(k == 0), stop=(k == KT - 1))
                zst = zsp.tile([128, Dt], bf16, tag="zst", name="zst")
                nc.scalar.activation(zst[:, :Dr], psz[:], CPY,
                                     scale=dis_t[:, t:t + 1])
                nc.sync.dma_start(zs_own[li][t * 128:(t + 1) * 128, :Dr],
                                  zst[:, :Dr])

            def emit_ag(li, half):
                if half == 0:
                    nc.gpsimd.collective_compute(
                        "AllGather", bass.mybir.AluOpType.bypass,
                        replica_groups=RG,
                        ins=[zs_own[li][:HA, :]],
                        outs=[zs_fullA[li].opt()])
                else:
                    nc.gpsimd.collective_compute(
                        "AllGather", bass.mybir.AluOpType.bypass,
                        replica_groups=RG,
                        ins=[zs_own[li][HA:, :]],
                        outs=[zs_fullB[li].opt()])

            # weight tiles for every layer, loaded up front (small)
            wk = []
            for li in range(NL):
                KT = DIN // 128 if li == 0 else DRS[li - 1] // 128
                wkl = []
                for k in range(KT):
                    wt_ = wtsp.tile([128, DRS[li]], bf16, tag=f"wk{li}_{k}",
                                    name=f"wk{li}_{k}")
                    nc.sync.dma_start(wt_[:], W_d[li][k * 128:(k + 1) * 128, :])
                    wkl.append(wt_)
                wk.append(wkl)

            # ---- layer-0 z-phase + split AllGather ----
            for t in range(NWIN):
                emit_z_tile(0, t, ht_cur)
                if t == NWA - 1:
                    emit_ag(0, 0)
            emit_ag(0, 1)

            # per-(window,half) idx/call offsets (stream: w-major, A then B)
            goffs = {}
            coffs = {}
            go = 0
            ci = 0
            for w in range(NWIN):
                for hh in range(2):
                    goffs[(w, hh)] = go
                    coffs[(w, hh)] = ci
                    go += int(G[w, hh])
                    ci += _cdiv(int(G[w, hh]), GMAX)

            # ---- main loop: gather/aggregate layer li; z + AG of li+1
            #      interleaved so the collectives hide under the gathers.
            #      The first GATH_BUFS windows' A-half calls are emitted
            #      before any B-half call: the in-order gpsimd queue then
            #      generates A descriptors while the B-table AllGather is
            #      still in flight instead of stalling behind it. ----
            for li in range(NL):
                Dr = DRS[li]
                Dt = DTS[li]
                qrr = li * 3  # decorrelate queue phase across layers

                def emit_calls(w, half, wt, Gl):
                    nonlocal qrr
                    tbl = zs_fullA[li] if half == 0 else zs_fullB[li]
                    gcnt = int(G[w, half])
                    gbase = 0 if half == 0 else Gl
                    g0 = 0
                    ci2 = coffs[(w, half)]
                    while g0 < gcnt:
                        gc = min(GMAX, gcnt - g0)
                        reg = cnt_regs[qrr % 4]
                        nc.gpsimd.reg_load(reg, cnt_t[0:1, ci2:ci2 + 1])
                        nc.gpsimd.dma_gather(
                            wt[:, gbase + g0:gbase + g0 + gc, :],
                            tbl[:],
                            idx_t[:, (goffs[(w, half)] + g0) * 8:
                                  (goffs[(w, half)] + g0 + gc) * 8],
                            num_idxs=gc * 128,
                            num_idxs_reg=reg,
                            elem_size=Dt,
                            queue_num=qrr % 4,
                        )
                        qrr += 1
                        ci2 += 1
                        g0 += gc

                PRE = min(GATH_BUFS, NWIN)
                tiles = {}
                for w in range(PRE):
                    wt = gathp.tile([128, G_CAP, Dt], bf16, tag="gather",
                                    name="wt")
                    if li == 0:
                        # first pool rotation: clear so slots skipped by
                        # short gathers never hold NaN bit patterns
                        nc.vector.memset(wt[:], 0.0)
                    tiles[w] = wt
                    emit_calls(w, 0, wt, int(G[w, 0]))
                for w in range(NWIN):
                    Gl, Gh = int(G[w, 0]), int(G[w, 1])
                    Gt = Gl + Gh
                    assert Gt > 0, "empty window unsupported"
                    if w < PRE:
                        wt = tiles.pop(w)
                    else:
                        wt = gathp.tile([128, G_CAP, Dt], bf16, tag="gather",
                                        name="wt")
                        emit_calls(w, 0, wt, Gl)
                    emit_calls(w, 1, wt, Gl)
                    # one-hot: oh[p, j, g] = (dstl[p, g] == j)
                    g_off = goffs[(w, 0)]
                    oh = ohp.tile([128, 128, Gt], bf16, tag="oh", name="oh")
                    nc.vector.tensor_tensor(
                        oh[:],
                        dstl_t[:, g_off:g_off + Gt].unsqueeze(1)
                            .broadcast_to((128, 128, Gt)),
                        iotag_t[:].rearrange("p (j g) -> p j g", g=G_CAP)
                            [:, :, :Gt],
                        op=EQ,
                    )
                    # z rows for the self term (bf16, local DRAM)
                    zw = zsp.tile([128, Dr], bf16, tag="zw", name="zw")
                    nc.scalar.dma_start(
                        zw[:], zs_own[li][w * 128:(w + 1) * 128, :Dr])
                    psw = pswp.tile([128, Dt], f32, tag="psw", name="psw")
                    for g in range(Gt):
                        nc.tensor.matmul(psw[:], oh[:, :, g], wt[:, g, :],
                                         start=(g == 0), stop=False)
                    # self term: psw[j] += zs_own[w*128+j] (epilogue's dis
                    # scale turns this into the dis^2 z contribution)
                    nc.tensor.matmul(psw[:, :Dr], ident_t[:], zw[:],
                                     start=False, stop=True)
                    if bias_t[li] is not None:
                        t1 = epip.tile([128, Dr], f32, tag="t1", name="t1")
                        nc.vector.tensor_tensor(t1[:], psw[:, :Dr],
                                                bias_t[li][:], op=ADD)
                        src_ap = t1[:]
                    else:
                        src_ap = psw[:, :Dr]
                    if li < NL - 1:
                        h2 = epip.tile([128, Dr], bf16, tag="h2", name="h2")
                        nc.scalar.activation(
                            h2[:], src_ap,
                            bass.mybir.ActivationFunctionType.Relu,
                            scale=dis_t[:, w:w + 1])
                        for k in range(Dr // 128):
                            pst = pstp.tile([128, 128], bf16, tag="pst",
                                            name="pst")
                            nc.tensor.transpose(
                                pst[:], h2[:, k * 128:(k + 1) * 128],
                                ident_t[:])
                            nc.vector.tensor_copy(
                                ht_nxt[k][:, w * 128:(w + 1) * 128], pst[:])
                        # interleaved z for layer li+1 (its H^T tile-w is
                        # ready now); AG halves fire mid-phase
                        emit_z_tile(li + 1, w, ht_nxt)
                        if w == NWA - 1:
                            emit_ag(li + 1, 0)
                    else:
                        h2o = epip.tile([128, Dr], f32, tag="h2o", name="h2o")
                        nc.scalar.activation(h2o[:], src_ap, CPY,
                                             scale=dis_t[:, w:w + 1])
                        nc.sync.dma_start(out_d[w * 128:(w + 1) * 128, :],
                                          h2o[:])
                if li < NL - 1:
                    emit_ag(li + 1, 1)
                    ht_cur, ht_nxt = ht_nxt, ht_cur
    nc.compile()
    return nc


# ---------------------------------------------------------------------------
# Entry point
# ---------------------------------------------------------------------------
def kernel(x, edge_index, W1, b1, W2, b2, W3, b3):
    from concourse.bass_utils import run_bass_kernel_spmd
    import ml_dtypes

    bfnp = ml_dtypes.bfloat16
    x = np.asarray(x, dtype=np.float32)
    Ws = [np.asarray(w, dtype=np.float32) for w in (W1, W2, W3)]
    bs = [np.asarray(b, dtype=np.float32) for b in (b1, b2, b3)]

    N, DIN = x.shape
    DRS = [w.shape[1] for w in Ws]
    DTS = [max(d, 128) for d in DRS]
    NL = 3

    dis, G, cores, CH, NWIN, CHP, NWA, NCALLS = preprocess(edge_index, N)
    TOT_IDX = cores[0]["idx"].shape[1] * 16
    TOT_G = cores[0]["dstl"].shape[1]
    G_CAP = int((G[:, 0] + G[:, 1]).max())
    biases_nonzero = [bool(np.any(b != 0)) for b in bs]

    nc = build_program(DIN, DRS, DTS, G, NWIN, CHP, NWA, TOT_IDX, TOT_G,
                       G_CAP, NCALLS, biases_nonzero)

    ident = np.eye(128, dtype=bfnp)
    # iotag[p, j*G_CAP + g] = j
    iotag = np.tile(np.repeat(np.arange(128), G_CAP).astype(bfnp), (128, 1))
    in_maps = []
    for c in range(NC_CORES):
        xT = np.zeros((DIN, CHP), bfnp)
        xT[:, :CH] = x[c * CH:(c + 1) * CH].T.astype(bfnp)
        m = {
            "xT": xT,
            "idx": cores[c]["idx"],
            "dstl": cores[c]["dstl"],
            "iotag": iotag,
            "dis_win": cores[c]["dis_win"],
            "ident": ident,
            "ncounts": cores[c]["ncounts"][None, :],
        }
        for i in range(NL):
            m[f"W{i}"] = Ws[i].astype(bfnp)
            m[f"bias{i}"] = np.tile(bs[i][None, :], (128, 1))
        in_maps.append(m)

    trace = bool(int(os.environ.get("GCN_TRACE", "0")))
    res = run_bass_kernel_spmd(nc, in_maps, core_ids=list(range(NC_CORES)),
                               trace=trace)
    kernel.last_results = res
    out = np.concatenate([res.results[c]["out"][:CH] for c in range(NC_CORES)],
                         axis=0)
    return out.astype(np.float32)



# revision 29
# speedup vs baseline: 1.0246x; 1.0188x over previous
"""Trainium2 Bass kernel for a 3-layer GCN (nn_BaselineGCN).

Strategy (8 NeuronCores, node partitioning by dst):
  - Host: compute deg/dis, partition edges by owner of dst (6250 nodes/core,
    padded to 6272), sort by (dst-window, src-half), build int16 gather
    indices (full node table split into two 25088-row halves so indices fit
    int16) plus per-edge local-dst values for one-hot construction.
  - Device, per layer (bf16 tables, fp32 accumulation):
      * data-parallel matmul  Zs_own = dis ⊙ (H_own @ W)        (TensorE)
      * AllGather Zs chunks -> full 50176-row bf16 table in HBM (collective)
      * per 128-dst window: dma_gather source rows (memory-bound part),
        one-hot(dst_local) built on DVE via is_equal vs iota, segment-sum
        via PE matmul accumulation in PSUM with one extra I @ Zs_own_w
        matmul for the (A+I) self term, epilogue relu(dis ⊙ acc + b),
        TensorE transpose -> next H^T kept resident in SBUF.
  - Layer 3 (64 outputs) runs on a 128-wide bf16 table (upper 64 cols
    garbage, excluded by slicing) so the whole edge path is uniform bf16.
"""
import sys
import os

sys.path.insert(0, "/opt/trn_rl_repo")

import numpy as np

NC_CORES = 8
GMAX = 8  # max groups (=1024 indices) per dma_gather call
GATH_BUFS = 9  # gather-tile pool depth (first GATH_BUFS windows are memset)


def _cdiv(a, b):
    return (a + b - 1) // b


# ---------------------------------------------------------------------------
# Host-side preprocessing
# ---------------------------------------------------------------------------
def preprocess(edge_index, N):
    src = np.asarray(edge_index[0], dtype=np.int64)
    dst = np.asarray(edge_index[1], dtype=np.int64)
    deg = np.bincount(dst, minlength=N).astype(np.float32) + np.float32(1.0)
    dis = (np.float32(1.0) / np.sqrt(deg)).astype(np.float32)

    CH = N // NC_CORES
    NWIN = _cdiv(CH, 128)
    CHP = NWIN * 128
    # split each core's chunk into A (windows 0..NWA-1) and B (the rest) so
    # the AllGather of A can fire mid-phase; int16 gather indices address
    # each half-table separately.
    # A as large as int16 gather indices allow (NC*HA <= 32768) so the
    # exposed tail AllGather over B is as small as possible
    NWA = min(NWIN - 1, 32768 // (NC_CORES * 128)) if NWIN > 1 else NWIN
    HA = NWA * 128            # rows per core in table A
    HB = CHP - HA             # rows per core in table B (may be 0 if NWIN==1)
    src_c = src // CH         # owning core of each src node
    src_o = src % CH          # offset within core

    counts = np.zeros((NC_CORES, NWIN, 2), dtype=np.int64)
    percore = []
    for c in range(NC_CORES):
        sel = (dst >= c * CH) & (dst < (c + 1) * CH)
        sc, so = src_c[sel], src_o[sel]
        ed = dst[sel] - c * CH
        w = ed >> 7
        h = (so >= HA).astype(np.int64)
        eidx = np.where(h == 0, sc * HA + so, sc * HB + (so - HA))
        order = np.lexsort((ed, h, w))
        eidx, ed, w, h = eidx[order], ed[order], w[order], h[order]
        np.add.at(counts[c], (w, h), 1)
        percore.append((eidx, ed, w, h))

    G = _cdiv(counts, 128).max(axis=0)  # [NWIN, 2]

    import ml_dtypes

    # shared call schedule: per (window, half), gather calls of <=GMAX groups
    calls = []  # (wi, hi, g0, gc)
    for wi in range(NWIN):
        for hi in range(2):
            g0 = 0
            while g0 < G[wi, hi]:
                gc = min(GMAX, G[wi, hi] - g0)
                calls.append((wi, hi, g0, gc))
                g0 += gc

    cores = []
    for c in range(NC_CORES):
        eidx, ed, w, h = percore[c]
        idx_parts, dstl_parts = [], []
        pos = 0
        for wi in range(NWIN):
            for hi in range(2):
                n = counts[c, wi, hi]
                g = G[wi, hi]
                seg_idx = np.full(g * 128, -1, dtype=np.int16)
                seg_dstl = np.full(g * 128, 255.0, dtype=np.float32)
                if n:
                    seg_idx[:n] = eidx[pos:pos + n].astype(np.int16)
                    seg_dstl[:n] = (ed[pos:pos + n] - wi * 128).astype(np.float32)
                    pos += n
                idx_parts.append(seg_idx)
                dstl_parts.append(seg_dstl)
        idx_all = np.concatenate(idx_parts)
        dstl_all = np.concatenate(dstl_parts)
        TOT_G = len(idx_all) // 128

        # per-call valid counts; empty calls keep one dummy idx (0) because a
        # zero-valid gather is undefined
        seg_base = {}
        pos2 = 0
        for wi in range(NWIN):
            for hi in range(2):
                seg_base[(wi, hi)] = pos2
                pos2 += G[wi, hi] * 128
        ncounts = np.zeros(len(calls), dtype=np.int32)
        for k, (wi, hi, g0, gc) in enumerate(calls):
            n = int(counts[c, wi, hi])
            v = min(max(n - g0 * 128, 0), gc * 128)
            if v == 0:
                idx_all[seg_base[(wi, hi)] + g0 * 128] = 0
                v = 1
            ncounts[k] = v

        # device layouts
        idx_tiled = np.tile(idx_all.reshape(-1, 16).T, (8, 1)).copy()
        # dstl: [128 edge-slot, TOT_G] bf16
        dstl_tiled = np.ascontiguousarray(
            dstl_all.reshape(TOT_G, 128).T).astype(ml_dtypes.bfloat16)
        d = np.ones(CHP, np.float32)
        d[:CH] = dis[c * CH:(c + 1) * CH]
        dis_win = np.ascontiguousarray(d.reshape(NWIN, 128).T)
        cores.append(dict(idx=idx_tiled, dstl=dstl_tiled, dis_win=dis_win,
                          ncounts=ncounts))
    return dis, G, cores, CH, NWIN, CHP, NWA, len(calls)


# ---------------------------------------------------------------------------
# Bass program
# ---------------------------------------------------------------------------
def build_program(DIN, DRS, DTS, G, NWIN, CHP, NWA, TOT_IDX, TOT_G,
                  G_CAP, NCALLS, biases_nonzero):
    """DRS: real per-layer output dims [256,256,64];
    DTS: padded table dims [256,256,128]."""
    from concourse import bacc, bass, tile, mybir

    f32 = mybir.dt.float32
    bf16 = mybir.dt.bfloat16
    i16 = mybir.dt.int16
    ADD = mybir.AluOpType.add
    EQ = mybir.AluOpType.is_equal
    CPY = mybir.ActivationFunctionType.Copy
    NL = len(DRS)

    nc = bacc.Bacc("TRN2", target_bir_lowering=False, debug=False,
                   enable_asserts=False, num_devices=NC_CORES,
                   num_swdge_queues=4, dynamic_dma_scratch_size=32768)

    # --- I/O tensors ---
    xT_d = nc.dram_tensor("xT", [DIN, CHP], bf16, kind="ExternalInput")
    W_d = [nc.dram_tensor(f"W{i}", [DRS[i - 1] if i else DIN, DRS[i]], bf16,
                          kind="ExternalInput") for i in range(NL)]
    bias_d = [nc.dram_tensor(f"bias{i}", [128, DRS[i]], f32,
                             kind="ExternalInput") for i in range(NL)]
    idx_d = nc.dram_tensor("idx", [128, TOT_IDX // 16], i16, kind="ExternalInput")
    dstl_d = nc.dram_tensor("dstl", [128, TOT_G], bf16, kind="ExternalInput")
    iotag_d = nc.dram_tensor("iotag", [128, 128 * G_CAP], bf16,
                             kind="ExternalInput")
    dis_d = nc.dram_tensor("dis_win", [128, NWIN], f32, kind="ExternalInput")
    ident_d = nc.dram_tensor("ident", [128, 128], bf16, kind="ExternalInput")
    cnt_d = nc.dram_tensor("ncounts", [1, NCALLS], mybir.dt.int32,
                           kind="ExternalInput")
    out_d = nc.dram_tensor("out", [CHP, DRS[-1]], f32, kind="ExternalOutput")

    with tile.TileContext(nc) as tc:
        with (
            tc.tile_pool(name="const", bufs=1) as constp,
            tc.tile_pool(name="ht", bufs=1) as htp,
            tc.tile_pool(name="wts", bufs=2) as wtsp,
            tc.tile_pool(name="zs", bufs=3) as zsp,
            tc.tile_pool(name="gath", bufs=GATH_BUFS) as gathp,
            tc.tile_pool(name="oh", bufs=4) as ohp,
            tc.tile_pool(name="epi", bufs=3) as epip,
            tc.tile_pool(name="psz", bufs=2, space="PSUM") as pszp,
            tc.tile_pool(name="psw", bufs=3, space="PSUM") as pswp,
            tc.tile_pool(name="pst", bufs=2, space="PSUM") as pstp,
            tc.tile_pool(name="dram", bufs=1, space="DRAM") as dramp,
        ):
            # --- persistent SBUF constants ---
            idx_t = constp.tile([128, TOT_IDX // 16], i16, tag="idx")
            nc.sync.dma_start(idx_t[:], idx_d[:])
            dstl_t = constp.tile([128, TOT_G], bf16, tag="dstl")
            nc.sync.dma_start(dstl_t[:], dstl_d[:])
            iotag_t = constp.tile([128, 128 * G_CAP], bf16, tag="iotag")
            nc.sync.dma_start(iotag_t[:], iotag_d[:])
            dis_t = constp.tile([128, NWIN], f32, tag="dis")
            nc.sync.dma_start(dis_t[:], dis_d[:])
            ident_t = constp.tile([128, 128], bf16, tag="ident")
            nc.sync.dma_start(ident_t[:], ident_d[:])
            cnt_t = constp.tile([1, NCALLS], mybir.dt.int32, tag="cnt")
            nc.sync.dma_start(cnt_t[:], cnt_d[:])
            cnt_regs = [nc.gpsimd.alloc_register(f"gcnt{i}") for i in range(4)]
            bias_t = []
            for i in range(NL):
                if biases_nonzero[i]:
                    bt = constp.tile([128, DRS[i]], f32, tag=f"bias{i}")
                    nc.sync.dma_start(bt[:], bias_d[i][:])
                    bias_t.append(bt)
                else:
                    bias_t.append(None)

            # --- H^T SBUF-resident double buffer: [k][128, CHP] bf16 ---
            KT0 = DIN // 128
            ht_cur = [htp.tile([128, CHP], bf16, tag=f"htA{k}",
                               name=f"htA{k}") for k in range(KT0)]
            XCH = _cdiv(CHP, 4 * 128) * 128
            for k in range(KT0):
                for x0 in range(0, CHP, XCH):
                    x1 = min(x0 + XCH, CHP)
                    nc.sync.dma_start(
                        ht_cur[k][:, x0:x1],
                        xT_d[k * 128:(k + 1) * 128, x0:x1])
            ht_nxt = [htp.tile([128, CHP], bf16, tag=f"htB{k}",
                               name=f"htB{k}") for k in range(KT0)]

            zs_own = [dramp.tile([CHP, DTS[i]], bf16, tag=f"zso{i}",
                                 name=f"zs_own{i}") for i in range(NL)]
            HA = NWA * 128
            HB = CHP - HA
            zs_fullA = [dramp.tile([NC_CORES * HA, DTS[i]], bf16,
                                   tag=f"zsfA{i}", addr_space="Shared",
                                   name=f"zs_fullA{i}") for i in range(NL)]
            zs_fullB = [dramp.tile([NC_CORES * HB, DTS[i]], bf16,
                                   tag=f"zsfB{i}", addr_space="Shared",
                                   name=f"zs_fullB{i}") for i in range(NL)]

            RG = [list(range(NC_CORES))]

            def emit_z_tile(li, t, lhs_tiles):
                """Z-matmul + scale for node-tile t of layer li."""
                Dr = DRS[li]
                Dt = DTS[li]
                KT = DIN // 128 if li == 0 else DRS[li - 1] // 128
                psz = pszp.tile([128, Dr], f32, tag="psz", name="psz")
                for k in range(KT):
                    nc.tensor.matmul(psz[:],
                                     lhs_tiles[k][:, t * 128:(t + 1) * 128],
                                     wk[li][k][:],
                                     start=(k == 0), stop=(k == KT - 1))
                zst = zsp.tile([128, Dt], bf16, tag="zst", name="zst")
                nc.scalar.activation(zst[:, :Dr], psz[:], CPY,
                                     scale=dis_t[:, t:t + 1])
                nc.sync.dma_start(zs_own[li][t * 128:(t + 1) * 128, :Dr],
                                  zst[:, :Dr])

            def emit_ag(li, half):
                if half == 0:
                    nc.gpsimd.collective_compute(
                        "AllGather", bass.mybir.AluOpType.bypass,
                        replica_groups=RG,
                        ins=[zs_own[li][:HA, :]],
                        outs=[zs_fullA[li].opt()])
                else:
                    nc.gpsimd.collective_compute(
                        "AllGather", bass.mybir.AluOpType.bypass,
                        replica_groups=RG,
                        ins=[zs_own[li][HA:, :]],
                        outs=[zs_fullB[li].opt()])

            # weight tiles for every layer, loaded up front (small)
            wk = []
            for li in range(NL):
                KT = DIN // 128 if li == 0 else DRS[li - 1] // 128
                wkl = []
                for k in range(KT):
                    wt_ = wtsp.tile([128, DRS[li]], bf16, tag=f"wk{li}_{k}",
                                    name=f"wk{li}_{k}")
                    nc.sync.dma_start(wt_[:], W_d[li][k * 128:(k + 1) * 128, :])
                    wkl.append(wt_)
                wk.append(wkl)

            # ---- layer-0 z-phase + split AllGather ----
            for t in range(NWIN):
                emit_z_tile(0, t, ht_cur)
                if t == NWA - 1:
                    emit_ag(0, 0)
            emit_ag(0, 1)

            # per-(window,half) idx/call offsets (stream: w-major, A then B)
            goffs = {}
            coffs = {}
            go = 0
            ci = 0
            for w in range(NWIN):
                for hh in range(2):
                    goffs[(w, hh)] = go
                    coffs[(w, hh)] = ci
                    go += int(G[w, hh])
                    ci += _cdiv(int(G[w, hh]), GMAX)

            # ---- main loop: gather/aggregate layer li; z + AG of li+1
            #      interleaved so the collectives hide under the gathers.
            #      The first GATH_BUFS windows' A-half calls are emitted
            #      before any B-half call: the in-order gpsimd queue then
            #      generates A descriptors while the B-table AllGather is
            #      still in flight instead of stalling behind it. ----
            for li in range(NL):
                Dr = DRS[li]
                Dt = DTS[li]
                qrr = li * 3  # decorrelate queue phase across layers

                def emit_calls(w, half, wt, Gl):
                    nonlocal qrr
                    tbl = zs_fullA[li] if half == 0 else zs_fullB[li]
                    gcnt = int(G[w, half])
                    gbase = 0 if half == 0 else Gl
                    g0 = 0
                    ci2 = coffs[(w, half)]
                    while g0 < gcnt:
                        gc = min(GMAX, gcnt - g0)
                        reg = cnt_regs[qrr % 4]
                        nc.gpsimd.reg_load(reg, cnt_t[0:1, ci2:ci2 + 1])
                        nc.gpsimd.dma_gather(
                            wt[:, gbase + g0:gbase + g0 + gc, :],
                            tbl[:],
                            idx_t[:, (goffs[(w, half)] + g0) * 8:
                                  (goffs[(w, half)] + g0 + gc) * 8],
                            num_idxs=gc * 128,
                            num_idxs_reg=reg,
                            elem_size=Dt,
                            queue_num=qrr % 4,
                        )
                        qrr += 1
                        ci2 += 1
                        g0 += gc

                PRE = min(GATH_BUFS, NWIN)
                tiles = {}
                for w in range(PRE):
                    wt = gathp.tile([128, G_CAP, Dt], bf16, tag="gather",
                                    name="wt")
                    if li == 0:
                        # first pool rotation: clear so slots skipped by
                        # short gathers never hold NaN bit patterns
                        nc.vector.memset(wt[:], 0.0)
                    tiles[w] = wt
                    emit_calls(w, 0, wt, int(G[w, 0]))
                for w in range(NWIN):
                    Gl, Gh = int(G[w, 0]), int(G[w, 1])
                    Gt = Gl + Gh
                    assert Gt > 0, "empty window unsupported"
                    if w < PRE:
                        wt = tiles.pop(w)
                    else:
                        wt = gathp.tile([128, G_CAP, Dt], bf16, tag="gather",
                                        name="wt")
                        emit_calls(w, 0, wt, Gl)
                    emit_calls(w, 1, wt, Gl)
                    # one-hot: oh[p, j, g] = (dstl[p, g] == j)
                    g_off = goffs[(w, 0)]
                    oh = ohp.tile([128, 128, Gt], bf16, tag="oh", name="oh")
                    nc.vector.tensor_tensor(
                        oh[:],
                        dstl_t[:, g_off:g_off + Gt].unsqueeze(1)
                            .broadcast_to((128, 128, Gt)),
                        iotag_t[:].rearrange("p (j g) -> p j g", g=G_CAP)
                            [:, :, :Gt],
                        op=EQ,
                    )
                    # z rows for the self term (bf16, local DRAM)
                    zw = zsp.tile([128, Dr], bf16, tag="zw", name="zw")
                    nc.scalar.dma_start(
                        zw[:], zs_own[li][w * 128:(w + 1) * 128, :Dr])
                    psw = pswp.tile([128, Dt], f32, tag="psw", name="psw")
                    for g in range(Gt):
                        nc.tensor.matmul(psw[:], oh[:, :, g], wt[:, g, :],
                                         start=(g == 0), stop=False)
                    # self term: psw[j] += zs_own[w*128+j] (epilogue's dis
                    # scale turns this into the dis^2 z contribution)
                    nc.tensor.matmul(psw[:, :Dr], ident_t[:], zw[:],
                                     start=False, stop=True)
                    if bias_t[li] is not None:
                        t1 = epip.tile([128, Dr], f32, tag="t1", name="t1")
                        nc.vector.tensor_tensor(t1[:], psw[:, :Dr],
                                                bias_t[li][:], op=ADD)
                        src_ap = t1[:]
                    else:
                        src_ap = psw[:, :Dr]
                    if li < NL - 1:
                        h2 = epip.tile([128, Dr], bf16, tag="h2", name="h2")
                        nc.scalar.activation(
                            h2[:], src_ap,
                            bass.mybir.ActivationFunctionType.Relu,
                            scale=dis_t[:, w:w + 1])
                        for k in range(Dr // 128):
                            pst = pstp.tile([128, 128], bf16, tag="pst",
                                            name="pst")
                            nc.tensor.transpose(
                                pst[:], h2[:, k * 128:(k + 1) * 128],
                                ident_t[:])
                            nc.vector.tensor_copy(
                                ht_nxt[k][:, w * 128:(w + 1) * 128], pst[:])
                        # interleaved z for layer li+1 (its H^T tile-w is
                        # ready now); AG halves fire mid-phase
                        emit_z_tile(li + 1, w, ht_nxt)
                        if w == NWA - 1:
                            emit_ag(li + 1, 0)
                    else:
                        h2o = epip.tile([128, Dr], f32, tag="h2o", name="h2o")
                        nc.scalar.activation(h2o[:], src_ap, CPY,
                                             scale=dis_t[:, w:w + 1])
                        nc.sync.dma_start(out_d[w * 128:(w + 1) * 128, :],
                                          h2o[:])
                if li < NL - 1:
                    emit_ag(li + 1, 1)
                    ht_cur, ht_nxt = ht_nxt, ht_cur
    nc.compile()
    return nc


# ---------------------------------------------------------------------------
# Entry point
# ---------------------------------------------------------------------------
def kernel(x, edge_index, W1, b1, W2, b2, W3, b3):
    from concourse.bass_utils import run_bass_kernel_spmd
    import ml_dtypes

    bfnp = ml_dtypes.bfloat16
    x = np.asarray(x, dtype=np.float32)
    Ws = [np.asarray(w, dtype=np.float32) for w in (W1, W2, W3)]
    bs = [np.asarray(b, dtype=np.float32) for b in (b1, b2, b3)]

    N, DIN = x.shape
    DRS = [w.shape[1] for w in Ws]
    DTS = [max(d, 128) for d in DRS]
    NL = 3

    dis, G, cores, CH, NWIN, CHP, NWA, NCALLS = preprocess(edge_index, N)
    TOT_IDX = cores[0]["idx"].shape[1] * 16
    TOT_G = cores[0]["dstl"].shape[1]
    G_CAP = int((G[:, 0] + G[:, 1]).max())
    biases_nonzero = [bool(np.any(b != 0)) for b in bs]

    nc = build_program(DIN, DRS, DTS, G, NWIN, CHP, NWA, TOT_IDX, TOT_G,
                       G_CAP, NCALLS, biases_nonzero)

    ident = np.eye(128, dtype=bfnp)
    # iotag[p, j*G_CAP + g] = j
    iotag = np.tile(np.repeat(np.arange(128), G_CAP).astype(bfnp), (128, 1))
    in_maps = []
    for c in range(NC_CORES):
        xT = np.zeros((DIN, CHP), bfnp)
        xT[:, :CH] = x[c * CH:(c + 1) * CH].T.astype(bfnp)
        m = {
            "xT": xT,
            "idx": cores[c]["idx"],
            "dstl": cores[c]["dstl"],
            "iotag": iotag,
            "dis_win": cores[c]["dis_win"],
            "ident": ident,
            "ncounts": cores[c]["ncounts"][None, :],
        }
        for i in range(NL):
            m[f"W{i}"] = Ws[i].astype(bfnp)
            m[f"bias{i}"] = np.tile(bs[i][None, :], (128, 1))
        in_maps.append(m)

    trace = bool(int(os.environ.get("GCN_TRACE", "0")))
    res = run_bass_kernel_spmd(nc, in_maps, core_ids=list(range(NC_CORES)),
                               trace=trace)
    kernel.last_results = res
    out = np.concatenate([res.results[c]["out"][:CH] for c in range(NC_CORES)],
                         axis=0)
    return out.astype(np.float32)

